# revision 18
# baseline (speedup 1.0000x reference)
"""Causal self-attention (B=2, T=2048, C=1024, H=16) on 8 Trainium2 cores.

Sharding: data-parallel over batch (2) x tensor-parallel over heads (4 groups
of 4 heads). Core c handles batch b = c//4, head group g = c%4 (heads 4g..4g+3).
Each core computes its qkv column slice, full causal TxT attention for its 4
heads, and a partial row-parallel projection. Host sums the 4 partial proj
outputs per batch and adds b_proj.

Key performance structure (vs the straightforward two-phase version):
- the PE clock ramps with sustained execution (0.65 -> 1.2 -> 2.4 GHz after
  ~3us of gap-free streaming), so the whole kernel is ONE fused pipeline that
  keeps the PE instruction stream dense: per 512-row i-block ci, emit the qkv
  projections for t-slice ci, then attention for i-block ci, with the
  normalization + output-projection matmuls of earlier blocks woven into the
  score/AV stream as fillers wherever a cross-engine dependency would
  otherwise stall the PE
- score chunk jc's AV matmuls are emitted one step late (after the scores of
  jc+1) so the ACT-engine exp latency is hidden behind PE work
- x and all weights are DMA'd as bf16 (same 1 cycle/row PE rate as fp32r,
  half the HBM traffic / startup latency); on-chip intermediates stay fp32r
  except yT/wp (bf16) - final tolerance is 2e-2, measured error ~1e-3
- softmax denominator comes free from a ones-column embedded in the padded AV
  stationary; both heads' denominator rows are staged into one pre-zeroed
  tile and partition-broadcast with a single K=128 selection matmul, then
  inverted with the fast approximate DVE reciprocal (~5x cheaper than exact)
- PSUM is exactly 8 banks: two pools of [128,2,512]x2 buffers; one rotation
  serves scores / norm-broadcast / projection / qkv tiles, the other the AV
  accumulators
- PSUM->SBUF drains run on the otherwise-idle Pool engine (gpsimd); ACT does
  exp only (it is co-critical with the PE), DVE does bias adds / masks /
  reciprocal / yT scaling
"""

import sys

sys.path.insert(0, "/opt/trn_rl_repo")

import numpy as np

P = 128
T = 2048
C = 1024
D = 64
HPC = 4          # heads per core
HD = HPC * D     # 256 qkv columns per core
CC = C // P      # 8 contraction chunks
TC = T // P      # 16 t-chunks of 128
IC = T // 512    # 4 i-blocks of 512

_NC = None
LAST_RESULTS = None


def _build_nc():
    import concourse.mybir as mybir
    import concourse.tile as tile
    from concourse import bacc
    from contextlib import ExitStack

    dt = mybir.dt
    f32 = dt.float32
    f32r = dt.float32r
    bf16 = dt.bfloat16
    ALU = mybir.AluOpType
    ACTF = mybir.ActivationFunctionType

    nc = bacc.Bacc(
        "TRN2",
        target_bir_lowering=False,
        debug=False,
        enable_asserts=False,
        num_devices=8,
    )

    xT = nc.dram_tensor("xT", [C, T], bf16, kind="ExternalInput").ap()
    wq = nc.dram_tensor("wq", [P, CC, HD], bf16, kind="ExternalInput").ap()
    wk = nc.dram_tensor("wk", [P, CC, HD], bf16, kind="ExternalInput").ap()
    wv = nc.dram_tensor("wv", [P, CC, HD], bf16, kind="ExternalInput").ap()
    wp = nc.dram_tensor("wp", [P, 2, C], bf16, kind="ExternalInput").ap()
    bq = nc.dram_tensor("bq", [P, 2], f32, kind="ExternalInput").ap()
    bk = nc.dram_tensor("bk", [P, 2], f32, kind="ExternalInput").ap()
    bv = nc.dram_tensor("bv", [P, 2, 2, D], f32, kind="ExternalInput").ap()
    tri = nc.dram_tensor("tri", [P, P], bf16, kind="ExternalInput").ap()
    tri2 = nc.dram_tensor("tri2", [P, 2 * P], bf16, kind="ExternalInput").ap()
    sel = nc.dram_tensor("sel", [P, P], f32r, kind="ExternalInput").ap()
    vinit = nc.dram_tensor("vinit", [P, TC, 2, D], bf16, kind="ExternalInput").ap()
    out = nc.dram_tensor("out", [T, C], bf16, kind="ExternalOutput").ap()

    with tile.TileContext(nc) as tc, ExitStack() as ctx:
        persist = ctx.enter_context(tc.tile_pool(name="persist", bufs=1))
        xT_sb = persist.tile([P, CC, T], bf16, name="xTs")
        wq_sb = persist.tile([P, CC, HD], bf16, name="wqs")
        wk_sb = persist.tile([P, CC, HD], bf16, name="wks")
        wv_sb = persist.tile([P, CC, HD], bf16, name="wvs")
        wp_sb = persist.tile([P, 2, C], bf16, name="wps")
        qT_sb = persist.tile([P, 2, T], f32r, name="qT")   # [d%128, hp, t]
        kT_sb = persist.tile([P, 2, T], f32r, name="kT")
        v_sb = persist.tile([P, TC, 2, 2, P], bf16, name="v")
        yT_sb = persist.tile([P, 2, T], bf16, name="yT")
        tri_sb = persist.tile([P, P], bf16, name="tris")
        tri2_sb = persist.tile([P, 2 * P], bf16, name="tri2s")
        sel_sb = persist.tile([P, P], f32r, name="sels")
        dsb = persist.tile([P, 512], f32r, name="dsb")
        bq_sb = persist.tile([P, 2], f32, name="bqs")
        bk_sb = persist.tile([P, 2], f32, name="bks")
        bv_sb = persist.tile([P, 2, 2, D], f32, name="bvs")

        # ---- input DMA, ordered so first-needed data lands first; every
        # large tensor is split across several queue pushes for parallelism.
        # Big streams go on the sync+gpsimd rings; small constants ride the
        # otherwise-idle vector ring so they don't delay x/w.
        xTr = xT.rearrange("(o p) t -> p o t", p=P)
        wqr = wq
        wkr = wk
        wvr = wv
        nc.scalar.dma_start(bk_sb[:], bk)
        nc.scalar.dma_start(bq_sb[:], bq)
        nc.scalar.dma_start(bv_sb[:], bv)
        nc.scalar.dma_start(sel_sb[:], sel)
        for h in range(4):
            cs = slice(2 * h, 2 * h + 2)
            eng = nc.sync if h % 2 == 0 else nc.gpsimd
            eng.dma_start(wk_sb[:, cs, :], wkr[:, cs, :])
        for cc in range(CC):
            eng = nc.sync if cc % 2 == 0 else nc.gpsimd
            eng.dma_start(xT_sb[:, cc, 0:256], xTr[:, cc, 0:256])
            eng.dma_start(xT_sb[:, cc, 256:512], xTr[:, cc, 256:512])
        for h in range(4):
            cs = slice(2 * h, 2 * h + 2)
            eng = nc.sync if h % 2 == 0 else nc.gpsimd
            eng.dma_start(wv_sb[:, cs, :], wvr[:, cs, :])
        for h in range(4):
            cs = slice(2 * h, 2 * h + 2)
            eng = nc.sync if h % 2 == 0 else nc.gpsimd
            eng.dma_start(wq_sb[:, cs, :], wqr[:, cs, :])
        nc.scalar.dma_start(v_sb[:, :, :, 0, D:P], vinit)
        nc.scalar.dma_start(v_sb[:, :, :, 1, 0:D], vinit)
        nc.scalar.dma_start(tri_sb[:], tri)
        nc.scalar.dma_start(tri2_sb[:], tri2)
        for tsl in range(1, IC):
            for cc in range(CC):
                eng = nc.sync if cc % 2 == 0 else nc.gpsimd
                eng.dma_start(
                    xT_sb[:, cc, tsl * 512:(tsl + 1) * 512],
                    xTr[:, cc, tsl * 512:(tsl + 1) * 512],
                )
            if tsl == 1:
                nc.sync.dma_start(wp_sb[:, :, 0:512], wp[:, :, 0:512])
                nc.gpsimd.dma_start(wp_sb[:, :, 512:C], wp[:, :, 512:C])
        # zero the denominator staging tile once; each norm only rewrites
        # rows 0 and 64, all other rows must read 0 for the selection matmul
        nc.vector.tensor_scalar_mul(
            dsb[:, :], sel_sb[:, 0:1].to_broadcast([P, 512]), 0.0
        )

        with (
            tc.tile_pool(name="sp", bufs=2, space="PSUM") as sp,
            tc.tile_pool(name="avp", bufs=2, space="PSUM") as avp,
            tc.tile_pool(name="exp", bufs=4) as expool,
            tc.tile_pool(name="oth", bufs=4) as othpool,
        ):
            scale = float(D) ** -0.5
            pend_av = []    # up to two deferred AV pairs (2-deep lookahead)
            pend_norm = []  # (hp, i0, av) awaiting broadcast+reciprocal+scale
            pend_proj = []  # tj indices ready for projection

            def flush_av():
                if not pend_av:
                    return
                hp, jc, njc, av, ex, c0 = pend_av.pop(0)
                for hi in range(2):
                    nc.tensor.matmul(
                        av[:, hi, c0:512],
                        v_sb[:, jc, hp, hi, :],
                        ex[:, hi, c0:512],
                        start=(jc == 0),
                        stop=(jc == njc - 1),
                        skip_group_check=True,
                    )
                if jc == njc - 1:
                    # stage both heads' denominator rows (emitted by the
                    # ones-column of the AV stationary) into the pre-zeroed
                    # dsb; partition-aligned copies (Pool cannot read PSUM)
                    nc.vector.tensor_copy(dsb[D:D + 1, :], av[D:D + 1, 0, :])
                    nc.vector.tensor_copy(dsb[0:1, :], av[0:1, 1, :])

            def emit_s(ci, hp, jc):
                i0 = ci * 512
                diag = jc >= 4 * ci
                o = (jc - 4 * ci) if diag else 0
                c0 = 2 * P if diag and o == 3 else o * P
                sps = sp.tile([P, 2, 512], f32, tag="s")
                for hi in range(2):
                    bp = D * hi
                    nc.tensor.matmul(
                        sps[:, hi, c0:512],
                        kT_sb[bp:bp + D, hp, jc * P:(jc + 1) * P],
                        qT_sb[bp:bp + D, hp, i0 + c0:i0 + 512],
                        start=True,
                        stop=True,
                        skip_group_check=True,
                    )
                ex = expool.tile([P, 2, 512], bf16, tag="ex")
                nc.scalar.activation(
                    ex[:, :, c0:512], sps[:, :, c0:512], ACTF.Exp, scale=scale
                )
                if diag and o == 3:
                    # cols 256-383 fully masked, 384-511 triangular
                    nc.vector.tensor_tensor(
                        ex[:, :, c0:512],
                        ex[:, :, c0:512],
                        tri2_sb[:, None, :].to_broadcast([P, 2, 2 * P]),
                        ALU.mult,
                    )
                elif diag:
                    nc.vector.tensor_tensor(
                        ex[:, :, c0:c0 + P],
                        ex[:, :, c0:c0 + P],
                        tri_sb[:, None, :].to_broadcast([P, 2, P]),
                        ALU.mult,
                    )
                return ex, c0

            def emit_norm_mm(hp, i0, av):
                # partition-broadcast both denominator rows with one K=128
                # selection matmul, invert with the fast approx reciprocal
                # (denominators are >= ~1, well inside its safe range),
                # scale the AV values into yT
                bps = sp.tile([P, 2, 512], f32, tag="s")
                nc.tensor.matmul(
                    bps[:, 0, :], sel_sb[:], dsb[:],
                    start=True, stop=True, skip_group_check=True,
                )
                rec = othpool.tile([P, 512], f32, tag="rec")
                nc.vector.reciprocal_approx_fast(rec[:], bps[:, 0, :])
                nc.vector.tensor_tensor(
                    yT_sb[0:D, hp, i0:i0 + 512], av[0:D, 0, :], rec[0:D, :],
                    ALU.mult,
                )
                nc.vector.tensor_tensor(
                    yT_sb[D:P, hp, i0:i0 + 512], av[D:P, 1, :], rec[D:P, :],
                    ALU.mult,
                )

            def emit_proj_unit(tj, tail=False):
                pps = sp.tile([P, 2, 512], f32, tag="s")
                ot = othpool.tile([P, C], bf16, tag="ot")
                for co in range(2):
                    for dc in range(2):
                        nc.tensor.matmul(
                            pps[:, co, :],
                            yT_sb[:, dc, tj * P:(tj + 1) * P],
                            wp_sb[:, dc, co * 512:(co + 1) * 512],
                            start=(dc == 0),
                            stop=(dc == 1),
                        )
                nc.vector.tensor_copy(
                    ot[:].rearrange("p (co n) -> p co n", co=2), pps[:]
                )
                # two stores per unit on alternating rings: halves the
                # per-queue drain (descriptor-rate-bound) without flooding
                # the rings with triggers; tail units split 4-way since
                # nothing overlaps the final drain
                three = (nc.sync, nc.gpsimd, nc.scalar)
                rings = ((three[tj % 3], three[(tj + 1) % 3])
                         if tail else (nc.sync, nc.gpsimd))
                for ph in range(2):
                    rings[ph].dma_start(
                        out[tj * P + ph * D:tj * P + (ph + 1) * D, :],
                        ot[ph * D:(ph + 1) * D, :],
                    )

            for ci in range(IC):
                # ---- qkv projections for t-slice ci ----
                # group order k -> v -> q: each group's PSUM drains on DVE
                # while the next group's matmuls stream, and the S(0) gate
                # (q bias add) is emitted per-co right behind its matmuls
                vdrains = []
                for gi, (w_s, b_s, dest) in enumerate(
                    ((wk_sb, bk_sb, kT_sb), (wq_sb, bq_sb, qT_sb))
                ):
                    ps = (avp if gi == 0 else sp).tile(
                        [P, 2, 512], f32, tag="av" if gi == 0 else "s"
                    )
                    for co in range(2):
                        for nh in range(2 if ci == 0 else 1):
                            n0 = ci * 512 + nh * 256
                            nw = 256 if ci == 0 else 512
                            for cc in range(CC):
                                nc.tensor.matmul(
                                    ps[:, co, nh * 256:nh * 256 + nw],
                                    w_s[:, cc, co * P:(co + 1) * P],
                                    xT_sb[:, cc, n0:n0 + nw],
                                    start=(cc == 0),
                                    stop=(cc == CC - 1),
                                    skip_group_check=True,
                                )
                        if gi == 0:
                            # previous i-block's trailing AV pairs: their
                            # exps have had a full matmul group to complete
                            flush_av()
                        nc.vector.tensor_tensor(
                            dest[:, co, ci * 512:(ci + 1) * 512],
                            ps[:, co, :],
                            b_s[:, co:co + 1].to_broadcast([P, 512]),
                            ALU.add,
                        )
                    if gi == 0:
                        ps = sp.tile([P, 2, 512], f32, tag="s")
                        for tj4 in range(4):
                            tj = 4 * ci + tj4
                            pr = ps[:, tj4 // 2,
                                    (tj4 % 2) * 256:(tj4 % 2) * 256 + 256]
                            for cc in range(CC):
                                nc.tensor.matmul(
                                    pr,
                                    xT_sb[:, cc, tj * P:(tj + 1) * P],
                                    wv_sb[:, cc, :],
                                    start=(cc == 0),
                                    stop=(cc == CC - 1),
                                    skip_group_check=True,
                                )
                            psv = pr.rearrange(
                                "p (hp hi d) -> p hp hi d", hi=2, d=D
                            )
                            vdrains.append((tj, psv))
                # v drains go on DVE after the q bias adds: their AV
                # consumers are the diagonal chunks, several jc away
                for tj, psv in vdrains:
                    nc.vector.tensor_tensor(
                        v_sb[:, tj, :, 0, 0:D], psv[:, :, 0, :], bv_sb[:, :, 0, :],
                        ALU.add,
                    )
                    nc.vector.tensor_tensor(
                        v_sb[:, tj, :, 1, D:P], psv[:, :, 1, :], bv_sb[:, :, 1, :],
                        ALU.add,
                    )
                if pend_norm:
                    emit_norm_mm(*pend_norm.pop(0))
                    if ci >= 1:
                        pend_proj.extend(range(4 * (ci - 1), 4 * (ci - 1) + 4))

                # ---- attention for i-block ci ----
                njc = 4 * (ci + 1)
                for hp in range(2):
                    av = avp.tile([P, 2, 512], f32, tag="av")
                    for jc in range(njc):
                        if len(pend_av) >= 2:
                            flush_av()
                        ex, c0 = emit_s(ci, hp, jc)
                        pend_av.append((hp, jc, njc, av, ex, c0))
                        if jc == 3 and pend_norm:
                            emit_norm_mm(*pend_norm.pop(0))
                        pslots = {8: (4, 6), 12: (6, 10), 16: (7, 13)}
                        if jc in pslots.get(njc, ()) and pend_proj:
                            emit_proj_unit(pend_proj.pop(0))
                    pend_norm.append((hp, ci * 512, av))

            while pend_av:
                flush_av()
            while pend_norm:
                emit_norm_mm(*pend_norm.pop(0))
            pend_proj.extend(range(4 * (IC - 1), 4 * (IC - 1) + 4))
            while pend_proj:
                emit_proj_unit(pend_proj.pop(0), tail=True)
    nc.compile()
    return nc


def _get_nc():
    global _NC
    if _NC is None:
        _NC = _build_nc()
    return _NC


def _in_maps(x, W_qkv, b_qkv, W_proj):
    import ml_dtypes

    bf16 = ml_dtypes.bfloat16
    tri = np.ascontiguousarray(np.triu(np.ones((P, P), dtype=np.float32)).astype(bf16))
    tri2 = np.ascontiguousarray(
        np.concatenate([np.zeros((P, P), np.float32),
                        np.triu(np.ones((P, P), np.float32))], axis=1).astype(bf16)
    )
    sel = np.zeros((P, P), dtype=np.float32)
    sel[D, 0:D] = 1.0
    sel[0, D:P] = 1.0
    vinit = np.zeros((P, TC, 2, D), dtype=bf16)
    vinit[:, :, :, 0] = 1.0
    in_maps = []
    for c in range(8):
        b, g = divmod(c, 4)
        s = slice(HD * g, HD * g + HD)
        sk = slice(C + HD * g, C + HD * g + HD)
        sv = slice(2 * C + HD * g, 2 * C + HD * g + HD)
        in_maps.append({
            "xT": np.ascontiguousarray(x[b].T.astype(bf16)),
            "wq": np.ascontiguousarray(
                W_qkv[:, s].reshape(CC, P, HD).transpose(1, 0, 2).astype(bf16)
            ),
            "wk": np.ascontiguousarray(
                W_qkv[:, sk].reshape(CC, P, HD).transpose(1, 0, 2).astype(bf16)
            ),
            "wv": np.ascontiguousarray(
                W_qkv[:, sv].reshape(CC, P, HD).transpose(1, 0, 2).astype(bf16)
            ),
            "wp": np.ascontiguousarray(
                W_proj[s, :].reshape(2, P, C).transpose(1, 0, 2).astype(bf16)
            ),
            "bq": np.ascontiguousarray(b_qkv[s].reshape(2, P).T),
            "bk": np.ascontiguousarray(b_qkv[sk].reshape(2, P).T),
            "bv": np.ascontiguousarray(
                np.broadcast_to(b_qkv[sv].reshape(2, 2, D), (P, 2, 2, D))
            ),
            "tri": tri,
            "tri2": tri2,
            "sel": sel,
            "vinit": vinit,
        })
    return in_maps


def kernel(x, W_qkv, b_qkv, W_proj, b_proj):
    global LAST_RESULTS
    from concourse import bass_utils

    x = np.asarray(x, dtype=np.float32)
    W_qkv = np.asarray(W_qkv, dtype=np.float32)
    b_qkv = np.asarray(b_qkv, dtype=np.float32)
    W_proj = np.asarray(W_proj, dtype=np.float32)
    b_proj = np.asarray(b_proj, dtype=np.float32)

    nc = _get_nc()
    in_maps = _in_maps(x, W_qkv, b_qkv, W_proj)
    res = bass_utils.run_bass_kernel_spmd(nc, in_maps, core_ids=list(range(8)))
    LAST_RESULTS = res
    ys = []
    for b in range(2):
        y = res.results[4 * b]["out"].astype(np.float64)
        for g in range(1, 4):
            y = y + res.results[4 * b + g]["out"]
        ys.append((y + b_proj).astype(np.float32))
    return np.stack(ys, axis=0)


# revision 19
# speedup vs baseline: 1.0070x; 1.0070x over previous
"""Causal self-attention (B=2, T=2048, C=1024, H=16) on 8 Trainium2 cores.

Sharding: data-parallel over batch (2) x tensor-parallel over heads (4 groups
of 4 heads). Core c handles batch b = c//4, head group g = c%4 (heads 4g..4g+3).
Each core computes its qkv column slice, full causal TxT attention for its 4
heads, and a partial row-parallel projection. Host sums the 4 partial proj
outputs per batch and adds b_proj.

Key performance structure (vs the straightforward two-phase version):
- the PE clock ramps with sustained execution (0.65 -> 1.2 -> 2.4 GHz after
  ~3us of gap-free streaming), so the whole kernel is ONE fused pipeline that
  keeps the PE instruction stream dense: per 512-row i-block ci, emit the qkv
  projections for t-slice ci, then attention for i-block ci, with the
  normalization + output-projection matmuls of earlier blocks woven into the
  score/AV stream as fillers wherever a cross-engine dependency would
  otherwise stall the PE
- score chunk jc's AV matmuls are emitted one step late (after the scores of
  jc+1) so the ACT-engine exp latency is hidden behind PE work
- x and all weights are DMA'd as bf16 (same 1 cycle/row PE rate as fp32r,
  half the HBM traffic / startup latency); on-chip intermediates stay fp32r
  except yT/wp (bf16) - final tolerance is 2e-2, measured error ~1e-3
- softmax denominator comes free from a ones-column embedded in the padded AV
  stationary; both heads' denominator rows are staged into one pre-zeroed
  tile and partition-broadcast with a single K=128 selection matmul, then
  inverted with the fast approximate DVE reciprocal (~5x cheaper than exact)
- PSUM is exactly 8 banks: two pools of [128,2,512]x2 buffers; one rotation
  serves scores / norm-broadcast / projection / qkv tiles, the other the AV
  accumulators
- PSUM->SBUF drains run on the otherwise-idle Pool engine (gpsimd); ACT does
  exp only (it is co-critical with the PE), DVE does bias adds / masks /
  reciprocal / yT scaling
"""

import sys

sys.path.insert(0, "/opt/trn_rl_repo")

import numpy as np

P = 128
T = 2048
C = 1024
D = 64
HPC = 4          # heads per core
HD = HPC * D     # 256 qkv columns per core
CC = C // P      # 8 contraction chunks
TC = T // P      # 16 t-chunks of 128
IC = T // 512    # 4 i-blocks of 512

_NC = None
LAST_RESULTS = None


def _build_nc():
    import concourse.mybir as mybir
    import concourse.tile as tile
    from concourse import bacc
    from contextlib import ExitStack

    dt = mybir.dt
    f32 = dt.float32
    f32r = dt.float32r
    bf16 = dt.bfloat16
    ALU = mybir.AluOpType
    ACTF = mybir.ActivationFunctionType

    nc = bacc.Bacc(
        "TRN2",
        target_bir_lowering=False,
        debug=False,
        enable_asserts=False,
        num_devices=8,
    )

    xT = nc.dram_tensor("xT", [C, T], bf16, kind="ExternalInput").ap()
    wq = nc.dram_tensor("wq", [P, CC, HD], bf16, kind="ExternalInput").ap()
    wk = nc.dram_tensor("wk", [P, CC, HD], bf16, kind="ExternalInput").ap()
    wv = nc.dram_tensor("wv", [P, CC, HD], bf16, kind="ExternalInput").ap()
    wp = nc.dram_tensor("wp", [P, 2, C], bf16, kind="ExternalInput").ap()
    bq = nc.dram_tensor("bq", [P, 2], f32, kind="ExternalInput").ap()
    bk = nc.dram_tensor("bk", [P, 2], f32, kind="ExternalInput").ap()
    bv = nc.dram_tensor("bv", [P, 2, 2, D], f32, kind="ExternalInput").ap()
    tri = nc.dram_tensor("tri", [P, P], bf16, kind="ExternalInput").ap()
    tri2 = nc.dram_tensor("tri2", [P, 2 * P], bf16, kind="ExternalInput").ap()
    sel = nc.dram_tensor("sel", [P, P], f32r, kind="ExternalInput").ap()
    vinit = nc.dram_tensor("vinit", [P, TC, 2, D], bf16, kind="ExternalInput").ap()
    out = nc.dram_tensor("out", [T, C], bf16, kind="ExternalOutput").ap()

    with tile.TileContext(nc) as tc, ExitStack() as ctx:
        persist = ctx.enter_context(tc.tile_pool(name="persist", bufs=1))
        xT_sb = persist.tile([P, CC, T], bf16, name="xTs")
        wq_sb = persist.tile([P, CC, HD], bf16, name="wqs")
        wk_sb = persist.tile([P, CC, HD], bf16, name="wks")
        wv_sb = persist.tile([P, CC, HD], bf16, name="wvs")
        wp_sb = persist.tile([P, 2, C], bf16, name="wps")
        qT_sb = persist.tile([P, 2, T], f32r, name="qT")   # [d%128, hp, t]
        kT_sb = persist.tile([P, 2, T], f32r, name="kT")
        v_sb = persist.tile([P, TC, 2, 2, P], bf16, name="v")
        yT_sb = persist.tile([P, 2, T], bf16, name="yT")
        tri_sb = persist.tile([P, P], bf16, name="tris")
        tri2_sb = persist.tile([P, 2 * P], bf16, name="tri2s")
        sel_sb = persist.tile([P, P], f32r, name="sels")
        dsb = persist.tile([P, 512], f32r, name="dsb")
        bq_sb = persist.tile([P, 2], f32, name="bqs")
        bk_sb = persist.tile([P, 2], f32, name="bks")
        bv_sb = persist.tile([P, 2, 2, D], f32, name="bvs")

        # ---- input DMA, ordered so first-needed data lands first; every
        # large tensor is split across several queue pushes for parallelism.
        # Big streams go on the sync+gpsimd rings; small constants ride the
        # otherwise-idle vector ring so they don't delay x/w.
        xTr = xT.rearrange("(o p) t -> p o t", p=P)
        wqr = wq
        wkr = wk
        wvr = wv
        nc.scalar.dma_start(bk_sb[:], bk)
        nc.scalar.dma_start(bq_sb[:], bq)
        nc.scalar.dma_start(bv_sb[:], bv)
        nc.scalar.dma_start(sel_sb[:], sel)
        for h in range(2):
            cs = slice(4 * h, 4 * h + 4)
            eng = nc.sync if h == 0 else nc.gpsimd
            eng.dma_start(wk_sb[:, cs, :], wkr[:, cs, :])
        for cc in range(CC):
            eng = nc.sync if cc % 2 == 0 else nc.gpsimd
            eng.dma_start(xT_sb[:, cc, 0:512], xTr[:, cc, 0:512])
        for h in range(2):
            cs = slice(4 * h, 4 * h + 4)
            eng = nc.sync if h == 0 else nc.gpsimd
            eng.dma_start(wv_sb[:, cs, :], wvr[:, cs, :])
        for h in range(2):
            cs = slice(4 * h, 4 * h + 4)
            eng = nc.sync if h == 0 else nc.gpsimd
            eng.dma_start(wq_sb[:, cs, :], wqr[:, cs, :])
        nc.scalar.dma_start(v_sb[:, :, :, 0, D:P], vinit)
        nc.scalar.dma_start(v_sb[:, :, :, 1, 0:D], vinit)
        nc.scalar.dma_start(tri_sb[:], tri)
        nc.scalar.dma_start(tri2_sb[:], tri2)
        for tsl in range(1, IC):
            for cc in range(CC):
                eng = nc.sync if cc % 2 == 0 else nc.gpsimd
                eng.dma_start(
                    xT_sb[:, cc, tsl * 512:(tsl + 1) * 512],
                    xTr[:, cc, tsl * 512:(tsl + 1) * 512],
                )
            if tsl == 1:
                nc.sync.dma_start(wp_sb[:, :, 0:512], wp[:, :, 0:512])
                nc.gpsimd.dma_start(wp_sb[:, :, 512:C], wp[:, :, 512:C])
        # zero the denominator staging tile once; each norm only rewrites
        # rows 0 and 64, all other rows must read 0 for the selection matmul
        nc.vector.tensor_scalar_mul(
            dsb[:, :], sel_sb[:, 0:1].to_broadcast([P, 512]), 0.0
        )

        with (
            tc.tile_pool(name="sp", bufs=2, space="PSUM") as sp,
            tc.tile_pool(name="avp", bufs=2, space="PSUM") as avp,
            tc.tile_pool(name="exp", bufs=4) as expool,
            tc.tile_pool(name="oth", bufs=4) as othpool,
        ):
            scale = float(D) ** -0.5
            pend_av = []    # up to two deferred AV pairs (2-deep lookahead)
            pend_norm = []  # (hp, i0, av) awaiting broadcast+reciprocal+scale
            pend_proj = []  # tj indices ready for projection

            def flush_av():
                if not pend_av:
                    return
                hp, jc, njc, av, ex, c0 = pend_av.pop(0)
                for hi in range(2):
                    nc.tensor.matmul(
                        av[:, hi, c0:512],
                        v_sb[:, jc, hp, hi, :],
                        ex[:, hi, c0:512],
                        start=(jc == 0),
                        stop=(jc == njc - 1),
                        skip_group_check=True,
                    )
                if jc == njc - 1:
                    # stage both heads' denominator rows (emitted by the
                    # ones-column of the AV stationary) into the pre-zeroed
                    # dsb; partition-aligned copies (Pool cannot read PSUM)
                    nc.vector.tensor_copy(dsb[D:D + 1, :], av[D:D + 1, 0, :])
                    nc.vector.tensor_copy(dsb[0:1, :], av[0:1, 1, :])

            def emit_s(ci, hp, jc):
                i0 = ci * 512
                diag = jc >= 4 * ci
                o = (jc - 4 * ci) if diag else 0
                c0 = 2 * P if diag and o == 3 else o * P
                sps = sp.tile([P, 2, 512], f32, tag="s")
                for hi in range(2):
                    bp = D * hi
                    nc.tensor.matmul(
                        sps[:, hi, c0:512],
                        kT_sb[bp:bp + D, hp, jc * P:(jc + 1) * P],
                        qT_sb[bp:bp + D, hp, i0 + c0:i0 + 512],
                        start=True,
                        stop=True,
                        skip_group_check=True,
                    )
                ex = expool.tile([P, 2, 512], bf16, tag="ex")
                nc.scalar.activation(
                    ex[:, :, c0:512], sps[:, :, c0:512], ACTF.Exp, scale=scale
                )
                if diag and o == 3:
                    # cols 256-383 fully masked, 384-511 triangular
                    nc.vector.tensor_tensor(
                        ex[:, :, c0:512],
                        ex[:, :, c0:512],
                        tri2_sb[:, None, :].to_broadcast([P, 2, 2 * P]),
                        ALU.mult,
                    )
                elif diag:
                    nc.vector.tensor_tensor(
                        ex[:, :, c0:c0 + P],
                        ex[:, :, c0:c0 + P],
                        tri_sb[:, None, :].to_broadcast([P, 2, P]),
                        ALU.mult,
                    )
                return ex, c0

            def emit_norm_mm(hp, i0, av):
                # partition-broadcast both denominator rows with one K=128
                # selection matmul, invert with the fast approx reciprocal
                # (denominators are >= ~1, well inside its safe range),
                # scale the AV values into yT
                bps = sp.tile([P, 2, 512], f32, tag="s")
                nc.tensor.matmul(
                    bps[:, 0, :], sel_sb[:], dsb[:],
                    start=True, stop=True, skip_group_check=True,
                )
                rec = othpool.tile([P, 512], f32, tag="rec")
                nc.vector.reciprocal_approx_fast(rec[:], bps[:, 0, :])
                nc.vector.tensor_tensor(
                    yT_sb[0:D, hp, i0:i0 + 512], av[0:D, 0, :], rec[0:D, :],
                    ALU.mult,
                )
                nc.vector.tensor_tensor(
                    yT_sb[D:P, hp, i0:i0 + 512], av[D:P, 1, :], rec[D:P, :],
                    ALU.mult,
                )

            def emit_proj_unit(tj, tail=False):
                pps = sp.tile([P, 2, 512], f32, tag="s")
                ot = othpool.tile([P, C], bf16, tag="ot")
                for co in range(2):
                    for dc in range(2):
                        nc.tensor.matmul(
                            pps[:, co, :],
                            yT_sb[:, dc, tj * P:(tj + 1) * P],
                            wp_sb[:, dc, co * 512:(co + 1) * 512],
                            start=(dc == 0),
                            stop=(dc == 1),
                        )
                nc.vector.tensor_copy(
                    ot[:].rearrange("p (co n) -> p co n", co=2), pps[:]
                )
                # two stores per unit on alternating rings: halves the
                # per-queue drain (descriptor-rate-bound) without flooding
                # the rings with triggers; tail units split 4-way since
                # nothing overlaps the final drain
                three = (nc.sync, nc.gpsimd, nc.scalar)
                rings = ((three[tj % 3], three[(tj + 1) % 3])
                         if tail else (nc.sync, nc.gpsimd))
                for ph in range(2):
                    rings[ph].dma_start(
                        out[tj * P + ph * D:tj * P + (ph + 1) * D, :],
                        ot[ph * D:(ph + 1) * D, :],
                    )

            for ci in range(IC):
                # ---- qkv projections for t-slice ci ----
                # group order k -> v -> q: each group's PSUM drains on DVE
                # while the next group's matmuls stream, and the S(0) gate
                # (q bias add) is emitted per-co right behind its matmuls
                vdrains = []
                for gi, (w_s, b_s, dest) in enumerate(
                    ((wk_sb, bk_sb, kT_sb), (wq_sb, bq_sb, qT_sb))
                ):
                    ps = (avp if gi == 0 else sp).tile(
                        [P, 2, 512], f32, tag="av" if gi == 0 else "s"
                    )
                    for co in range(2):
                        for cc in range(CC):
                            nc.tensor.matmul(
                                ps[:, co, :],
                                w_s[:, cc, co * P:(co + 1) * P],
                                xT_sb[:, cc, ci * 512:(ci + 1) * 512],
                                start=(cc == 0),
                                stop=(cc == CC - 1),
                            )
                        if gi == 0:
                            # previous i-block's trailing AV pairs: their
                            # exps have had a full matmul group to complete
                            flush_av()
                        nc.vector.tensor_tensor(
                            dest[:, co, ci * 512:(ci + 1) * 512],
                            ps[:, co, :],
                            b_s[:, co:co + 1].to_broadcast([P, 512]),
                            ALU.add,
                        )
                    if gi == 0:
                        ps = sp.tile([P, 2, 512], f32, tag="s")
                        for tj4 in range(4):
                            tj = 4 * ci + tj4
                            pr = ps[:, tj4 // 2,
                                    (tj4 % 2) * 256:(tj4 % 2) * 256 + 256]
                            for cc in range(CC):
                                nc.tensor.matmul(
                                    pr,
                                    xT_sb[:, cc, tj * P:(tj + 1) * P],
                                    wv_sb[:, cc, :],
                                    start=(cc == 0),
                                    stop=(cc == CC - 1),
                                    skip_group_check=True,
                                )
                            psv = pr.rearrange(
                                "p (hp hi d) -> p hp hi d", hi=2, d=D
                            )
                            vdrains.append((tj, psv))
                # v drains go on DVE after the q bias adds: their AV
                # consumers are the diagonal chunks, several jc away
                for tj, psv in vdrains:
                    nc.vector.tensor_tensor(
                        v_sb[:, tj, :, 0, 0:D], psv[:, :, 0, :], bv_sb[:, :, 0, :],
                        ALU.add,
                    )
                    nc.vector.tensor_tensor(
                        v_sb[:, tj, :, 1, D:P], psv[:, :, 1, :], bv_sb[:, :, 1, :],
                        ALU.add,
                    )
                if pend_norm:
                    emit_norm_mm(*pend_norm.pop(0))
                    if ci >= 1:
                        pend_proj.extend(range(4 * (ci - 1), 4 * (ci - 1) + 4))

                # ---- attention for i-block ci ----
                njc = 4 * (ci + 1)
                for hp in range(2):
                    av = avp.tile([P, 2, 512], f32, tag="av")
                    for jc in range(njc):
                        if len(pend_av) >= 2:
                            flush_av()
                        ex, c0 = emit_s(ci, hp, jc)
                        pend_av.append((hp, jc, njc, av, ex, c0))
                        if jc == 3 and pend_norm:
                            emit_norm_mm(*pend_norm.pop(0))
                        pslots = {8: (4, 6), 12: (6, 10), 16: (7, 13)}
                        if jc in pslots.get(njc, ()) and pend_proj:
                            emit_proj_unit(pend_proj.pop(0))
                    pend_norm.append((hp, ci * 512, av))

            while pend_av:
                flush_av()
            while pend_norm:
                emit_norm_mm(*pend_norm.pop(0))
            pend_proj.extend(range(4 * (IC - 1), 4 * (IC - 1) + 4))
            while pend_proj:
                emit_proj_unit(pend_proj.pop(0), tail=True)
    nc.compile()
    return nc


def _get_nc():
    global _NC
    if _NC is None:
        _NC = _build_nc()
    return _NC


def _in_maps(x, W_qkv, b_qkv, W_proj):
    import ml_dtypes

    bf16 = ml_dtypes.bfloat16
    tri = np.ascontiguousarray(np.triu(np.ones((P, P), dtype=np.float32)).astype(bf16))
    tri2 = np.ascontiguousarray(
        np.concatenate([np.zeros((P, P), np.float32),
                        np.triu(np.ones((P, P), np.float32))], axis=1).astype(bf16)
    )
    sel = np.zeros((P, P), dtype=np.float32)
    sel[D, 0:D] = 1.0
    sel[0, D:P] = 1.0
    vinit = np.zeros((P, TC, 2, D), dtype=bf16)
    vinit[:, :, :, 0] = 1.0
    in_maps = []
    for c in range(8):
        b, g = divmod(c, 4)
        s = slice(HD * g, HD * g + HD)
        sk = slice(C + HD * g, C + HD * g + HD)
        sv = slice(2 * C + HD * g, 2 * C + HD * g + HD)
        in_maps.append({
            "xT": np.ascontiguousarray(x[b].T.astype(bf16)),
            "wq": np.ascontiguousarray(
                W_qkv[:, s].reshape(CC, P, HD).transpose(1, 0, 2).astype(bf16)
            ),
            "wk": np.ascontiguousarray(
                W_qkv[:, sk].reshape(CC, P, HD).transpose(1, 0, 2).astype(bf16)
            ),
            "wv": np.ascontiguousarray(
                W_qkv[:, sv].reshape(CC, P, HD).transpose(1, 0, 2).astype(bf16)
            ),
            "wp": np.ascontiguousarray(
                W_proj[s, :].reshape(2, P, C).transpose(1, 0, 2).astype(bf16)
            ),
            "bq": np.ascontiguousarray(b_qkv[s].reshape(2, P).T),
            "bk": np.ascontiguousarray(b_qkv[sk].reshape(2, P).T),
            "bv": np.ascontiguousarray(
                np.broadcast_to(b_qkv[sv].reshape(2, 2, D), (P, 2, 2, D))
            ),
            "tri": tri,
            "tri2": tri2,
            "sel": sel,
            "vinit": vinit,
        })
    return in_maps


def kernel(x, W_qkv, b_qkv, W_proj, b_proj):
    global LAST_RESULTS
    from concourse import bass_utils

    x = np.asarray(x, dtype=np.float32)
    W_qkv = np.asarray(W_qkv, dtype=np.float32)
    b_qkv = np.asarray(b_qkv, dtype=np.float32)
    W_proj = np.asarray(W_proj, dtype=np.float32)
    b_proj = np.asarray(b_proj, dtype=np.float32)

    nc = _get_nc()
    in_maps = _in_maps(x, W_qkv, b_qkv, W_proj)
    res = bass_utils.run_bass_kernel_spmd(nc, in_maps, core_ids=list(range(8)))
    LAST_RESULTS = res
    ys = []
    for b in range(2):
        y = res.results[4 * b]["out"].astype(np.float64)
        for g in range(1, 4):
            y = y + res.results[4 * b + g]["out"]
        ys.append((y + b_proj).astype(np.float32))
    return np.stack(ys, axis=0)


# revision 20
# speedup vs baseline: 1.0194x; 1.0124x over previous
"""Causal self-attention (B=2, T=2048, C=1024, H=16) on 8 Trainium2 cores.

Sharding: data-parallel over batch (2) x tensor-parallel over heads (4 groups
of 4 heads). Core c handles batch b = c//4, head group g = c%4 (heads 4g..4g+3).
Each core computes its qkv column slice, full causal TxT attention for its 4
heads, and a partial row-parallel projection. Host sums the 4 partial proj
outputs per batch and adds b_proj.

Key performance structure (vs the straightforward two-phase version):
- the PE clock ramps with sustained execution (0.65 -> 1.2 -> 2.4 GHz after
  ~3us of gap-free streaming), so the whole kernel is ONE fused pipeline that
  keeps the PE instruction stream dense: per 512-row i-block ci, emit the qkv
  projections for t-slice ci, then attention for i-block ci, with the
  normalization + output-projection matmuls of earlier blocks woven into the
  score/AV stream as fillers wherever a cross-engine dependency would
  otherwise stall the PE
- score chunk jc's AV matmuls are emitted one step late (after the scores of
  jc+1) so the ACT-engine exp latency is hidden behind PE work
- x and all weights are DMA'd as bf16 (same 1 cycle/row PE rate as fp32r,
  half the HBM traffic / startup latency); on-chip intermediates stay fp32r
  except yT/wp (bf16) - final tolerance is 2e-2, measured error ~1e-3
- softmax denominator comes free from a ones-column embedded in the padded AV
  stationary; both heads' denominator rows are staged into one pre-zeroed
  tile and partition-broadcast with a single K=128 selection matmul, then
  inverted with the fast approximate DVE reciprocal (~5x cheaper than exact)
- PSUM is exactly 8 banks: two pools of [128,2,512]x2 buffers; one rotation
  serves scores / norm-broadcast / projection / qkv tiles, the other the AV
  accumulators
- PSUM->SBUF drains run on the otherwise-idle Pool engine (gpsimd); ACT does
  exp only (it is co-critical with the PE), DVE does bias adds / masks /
  reciprocal / yT scaling
"""

import sys

sys.path.insert(0, "/opt/trn_rl_repo")

import numpy as np

P = 128
T = 2048
C = 1024
D = 64
HPC = 4          # heads per core
HD = HPC * D     # 256 qkv columns per core
CC = C // P      # 8 contraction chunks
TC = T // P      # 16 t-chunks of 128
IC = T // 512    # 4 i-blocks of 512

_NC = None
LAST_RESULTS = None


def _build_nc():
    import concourse.mybir as mybir
    import concourse.tile as tile
    from concourse import bacc
    from contextlib import ExitStack

    dt = mybir.dt
    f32 = dt.float32
    f32r = dt.float32r
    bf16 = dt.bfloat16
    ALU = mybir.AluOpType
    ACTF = mybir.ActivationFunctionType

    nc = bacc.Bacc(
        "TRN2",
        target_bir_lowering=False,
        debug=False,
        enable_asserts=False,
        num_devices=8,
    )

    xT = nc.dram_tensor("xT", [C, T], bf16, kind="ExternalInput").ap()
    wq = nc.dram_tensor("wq", [P, CC, HD], bf16, kind="ExternalInput").ap()
    wk = nc.dram_tensor("wk", [P, CC, HD], bf16, kind="ExternalInput").ap()
    wv = nc.dram_tensor("wv", [P, CC, HD], bf16, kind="ExternalInput").ap()
    wp = nc.dram_tensor("wp", [P, 2, C], bf16, kind="ExternalInput").ap()
    bq = nc.dram_tensor("bq", [P, 2], f32, kind="ExternalInput").ap()
    bk = nc.dram_tensor("bk", [P, 2], f32, kind="ExternalInput").ap()
    bv = nc.dram_tensor("bv", [P, 2, 2, D], f32, kind="ExternalInput").ap()
    tri = nc.dram_tensor("tri", [P, P], bf16, kind="ExternalInput").ap()
    tri2 = nc.dram_tensor("tri2", [P, 2 * P], bf16, kind="ExternalInput").ap()
    sel = nc.dram_tensor("sel", [P, P], f32r, kind="ExternalInput").ap()
    vinit = nc.dram_tensor("vinit", [P, TC, 2, D], bf16, kind="ExternalInput").ap()
    out = nc.dram_tensor("out", [T, C], bf16, kind="ExternalOutput").ap()

    with tile.TileContext(nc) as tc, ExitStack() as ctx:
        persist = ctx.enter_context(tc.tile_pool(name="persist", bufs=1))
        xT_sb = persist.tile([P, CC, T], bf16, name="xTs")
        wq_sb = persist.tile([P, CC, HD], bf16, name="wqs")
        wk_sb = persist.tile([P, CC, HD], bf16, name="wks")
        wv_sb = persist.tile([P, CC, HD], bf16, name="wvs")
        wp_sb = persist.tile([P, 2, C], bf16, name="wps")
        qT_sb = persist.tile([P, 2, T], f32r, name="qT")   # [d%128, hp, t]
        kT_sb = persist.tile([P, 2, T], f32r, name="kT")
        v_sb = persist.tile([P, TC, 2, 2, P], bf16, name="v")
        yT_sb = persist.tile([P, 2, T], bf16, name="yT")
        tri_sb = persist.tile([P, P], bf16, name="tris")
        tri2_sb = persist.tile([P, 2 * P], bf16, name="tri2s")
        sel_sb = persist.tile([P, P], f32r, name="sels")
        dsb = persist.tile([P, 512], f32r, name="dsb")
        bq_sb = persist.tile([P, 2], f32, name="bqs")
        bk_sb = persist.tile([P, 2], f32, name="bks")
        bv_sb = persist.tile([P, 2, 2, D], f32, name="bvs")

        # ---- input DMA, ordered so first-needed data lands first; every
        # large tensor is split across several queue pushes for parallelism.
        # Big streams go on the sync+gpsimd rings; small constants ride the
        # otherwise-idle vector ring so they don't delay x/w.
        xTr = xT.rearrange("(o p) t -> p o t", p=P)
        wqr = wq
        wkr = wk
        wvr = wv
        nc.scalar.dma_start(bk_sb[:], bk)
        nc.scalar.dma_start(bq_sb[:], bq)
        nc.scalar.dma_start(bv_sb[:], bv)
        nc.scalar.dma_start(sel_sb[:], sel)
        for h in range(2):
            cs = slice(4 * h, 4 * h + 4)
            eng = nc.sync if h == 0 else nc.gpsimd
            eng.dma_start(wk_sb[:, cs, :], wkr[:, cs, :])
        for cc in range(CC):
            eng = nc.sync if cc % 2 == 0 else nc.gpsimd
            eng.dma_start(xT_sb[:, cc, 0:512], xTr[:, cc, 0:512])
        for h in range(2):
            cs = slice(4 * h, 4 * h + 4)
            eng = nc.sync if h == 0 else nc.gpsimd
            eng.dma_start(wv_sb[:, cs, :], wvr[:, cs, :])
        for h in range(2):
            cs = slice(4 * h, 4 * h + 4)
            eng = nc.sync if h == 0 else nc.gpsimd
            eng.dma_start(wq_sb[:, cs, :], wqr[:, cs, :])
        nc.scalar.dma_start(v_sb[:, :, :, 0, D:P], vinit)
        nc.scalar.dma_start(v_sb[:, :, :, 1, 0:D], vinit)
        nc.scalar.dma_start(tri_sb[:], tri)
        nc.scalar.dma_start(tri2_sb[:], tri2)
        for tsl in range(1, IC):
            for cc in range(CC):
                eng = nc.sync if cc % 2 == 0 else nc.gpsimd
                eng.dma_start(
                    xT_sb[:, cc, tsl * 512:(tsl + 1) * 512],
                    xTr[:, cc, tsl * 512:(tsl + 1) * 512],
                )
            if tsl == 1:
                nc.sync.dma_start(wp_sb[:, :, 0:512], wp[:, :, 0:512])
                nc.gpsimd.dma_start(wp_sb[:, :, 512:C], wp[:, :, 512:C])
        # zero the denominator staging tile once; each norm only rewrites
        # rows 0 and 64, all other rows must read 0 for the selection matmul
        nc.vector.tensor_scalar_mul(
            dsb[:, :], sel_sb[:, 0:1].to_broadcast([P, 512]), 0.0
        )

        with (
            tc.tile_pool(name="sp", bufs=2, space="PSUM") as sp,
            tc.tile_pool(name="avp", bufs=2, space="PSUM") as avp,
            tc.tile_pool(name="exp", bufs=4) as expool,
            tc.tile_pool(name="oth", bufs=4) as othpool,
        ):
            scale = float(D) ** -0.5
            pend_av = []    # up to two deferred AV pairs (2-deep lookahead)
            pend_norm = []  # (hp, i0, av) awaiting broadcast+reciprocal+scale
            pend_proj = []  # tj indices ready for projection

            def flush_av():
                if not pend_av:
                    return
                hp, jc, njc, av, ex, c0 = pend_av.pop(0)
                for hi in range(2):
                    nc.tensor.matmul(
                        av[:, hi, c0:512],
                        v_sb[:, jc, hp, hi, :],
                        ex[:, hi, c0:512],
                        start=(jc == 0),
                        stop=(jc == njc - 1),
                        skip_group_check=True,
                    )
                if jc == njc - 1:
                    # stage both heads' denominator rows (emitted by the
                    # ones-column of the AV stationary) into the pre-zeroed
                    # dsb; partition-aligned copies (Pool cannot read PSUM)
                    nc.vector.tensor_copy(dsb[D:D + 1, :], av[D:D + 1, 0, :])
                    nc.vector.tensor_copy(dsb[0:1, :], av[0:1, 1, :])

            def emit_s(ci, hp, jc):
                i0 = ci * 512
                diag = jc >= 4 * ci
                o = (jc - 4 * ci) if diag else 0
                c0 = 2 * P if diag and o == 3 else o * P
                sps = sp.tile([P, 2, 512], f32, tag="s")
                for hi in range(2):
                    bp = D * hi
                    nc.tensor.matmul(
                        sps[:, hi, c0:512],
                        kT_sb[bp:bp + D, hp, jc * P:(jc + 1) * P],
                        qT_sb[bp:bp + D, hp, i0 + c0:i0 + 512],
                        start=True,
                        stop=True,
                        skip_group_check=True,
                    )
                ex = expool.tile([P, 2, 512], bf16, tag="ex")
                nc.scalar.activation(
                    ex[:, :, c0:512], sps[:, :, c0:512], ACTF.Exp, scale=scale
                )
                if diag and o == 3:
                    # cols 256-383 fully masked, 384-511 triangular
                    nc.vector.tensor_tensor(
                        ex[:, :, c0:512],
                        ex[:, :, c0:512],
                        tri2_sb[:, None, :].to_broadcast([P, 2, 2 * P]),
                        ALU.mult,
                    )
                elif diag:
                    nc.vector.tensor_tensor(
                        ex[:, :, c0:c0 + P],
                        ex[:, :, c0:c0 + P],
                        tri_sb[:, None, :].to_broadcast([P, 2, P]),
                        ALU.mult,
                    )
                return ex, c0

            def emit_norm_mm(hp, i0, av):
                # partition-broadcast both denominator rows with one K=128
                # selection matmul, invert with the fast approx reciprocal
                # (denominators are >= ~1, well inside its safe range),
                # scale the AV values into yT
                bps = sp.tile([P, 2, 512], f32, tag="s")
                nc.tensor.matmul(
                    bps[:, 0, :], sel_sb[:], dsb[:],
                    start=True, stop=True, skip_group_check=True,
                )
                rec = othpool.tile([P, 512], f32, tag="rec")
                nc.vector.reciprocal_approx_fast(rec[:], bps[:, 0, :])
                nc.vector.tensor_tensor(
                    yT_sb[0:D, hp, i0:i0 + 512], av[0:D, 0, :], rec[0:D, :],
                    ALU.mult,
                )
                nc.vector.tensor_tensor(
                    yT_sb[D:P, hp, i0:i0 + 512], av[D:P, 1, :], rec[D:P, :],
                    ALU.mult,
                )

            def emit_proj_unit(tj, tail=False):
                pps = sp.tile([P, 2, 512], f32, tag="s")
                ot = othpool.tile([P, C], bf16, tag="ot")
                for co in range(2):
                    for dc in range(2):
                        nc.tensor.matmul(
                            pps[:, co, :],
                            yT_sb[:, dc, tj * P:(tj + 1) * P],
                            wp_sb[:, dc, co * 512:(co + 1) * 512],
                            start=(dc == 0),
                            stop=(dc == 1),
                        )
                nc.vector.tensor_copy(
                    ot[:].rearrange("p (co n) -> p co n", co=2), pps[:]
                )
                # two stores per unit on alternating rings: halves the
                # per-queue drain (descriptor-rate-bound) without flooding
                # the rings with triggers; tail units split 4-way since
                # nothing overlaps the final drain
                three = (nc.sync, nc.gpsimd, nc.scalar)
                rings = ((three[tj % 3], three[(tj + 1) % 3])
                         if tail else (nc.sync, nc.gpsimd))
                for ph in range(2):
                    rings[ph].dma_start(
                        out[tj * P + ph * D:tj * P + (ph + 1) * D, :],
                        ot[ph * D:(ph + 1) * D, :],
                    )

            for ci in range(IC):
                # ---- qkv projections for t-slice ci ----
                # group order k -> v -> q: each group's PSUM drains on DVE
                # while the next group's matmuls stream, and the S(0) gate
                # (q bias add) is emitted per-co right behind its matmuls
                vdrains = []
                for gi, (w_s, b_s, dest) in enumerate(
                    ((wk_sb, bk_sb, kT_sb), (wq_sb, bq_sb, qT_sb))
                ):
                    ps = (avp if gi == 0 else sp).tile(
                        [P, 2, 512], f32, tag="av" if gi == 0 else "s"
                    )
                    for co in range(2):
                        for cc in range(CC):
                            nc.tensor.matmul(
                                ps[:, co, :],
                                w_s[:, cc, co * P:(co + 1) * P],
                                xT_sb[:, cc, ci * 512:(ci + 1) * 512],
                                start=(cc == 0),
                                stop=(cc == CC - 1),
                            )
                        if gi == 0:
                            # previous i-block's trailing AV pairs: their
                            # exps have had a full matmul group to complete
                            flush_av()
                        nc.vector.tensor_tensor(
                            dest[:, co, ci * 512:(ci + 1) * 512],
                            ps[:, co, :],
                            b_s[:, co:co + 1].to_broadcast([P, 512]),
                            ALU.add,
                        )
                    if gi == 0:
                        ps = sp.tile([P, 2, 512], f32, tag="s")
                        for tj4 in range(4):
                            tj = 4 * ci + tj4
                            pr = ps[:, tj4 // 2,
                                    (tj4 % 2) * 256:(tj4 % 2) * 256 + 256]
                            for cc in range(CC):
                                nc.tensor.matmul(
                                    pr,
                                    xT_sb[:, cc, tj * P:(tj + 1) * P],
                                    wv_sb[:, cc, :],
                                    start=(cc == 0),
                                    stop=(cc == CC - 1),
                                    skip_group_check=True,
                                )
                            psv = pr.rearrange(
                                "p (hp hi d) -> p hp hi d", hi=2, d=D
                            )
                            vdrains.append((tj, psv))
                # v drains go on DVE after the q bias adds: their AV
                # consumers are the diagonal chunks, several jc away
                for tj, psv in vdrains:
                    nc.vector.tensor_tensor(
                        v_sb[:, tj, :, 0, 0:D], psv[:, :, 0, :], bv_sb[:, :, 0, :],
                        ALU.add,
                    )
                    nc.vector.tensor_tensor(
                        v_sb[:, tj, :, 1, D:P], psv[:, :, 1, :], bv_sb[:, :, 1, :],
                        ALU.add,
                    )
                if pend_norm:
                    emit_norm_mm(*pend_norm.pop(0))
                    if ci >= 1:
                        pend_proj.extend(range(4 * (ci - 1), 4 * (ci - 1) + 4))

                # ---- attention for i-block ci ----
                njc = 4 * (ci + 1)
                for hp in range(2):
                    av = avp.tile([P, 2, 512], f32, tag="av")
                    for jc in range(njc):
                        if len(pend_av) >= 2:
                            flush_av()
                        ex, c0 = emit_s(ci, hp, jc)
                        pend_av.append((hp, jc, njc, av, ex, c0))
                        if jc == 3 and pend_norm:
                            emit_norm_mm(*pend_norm.pop(0))
                        pslots = {8: (4, 6), 12: (5, 9), 16: (6, 11)}
                        if jc in pslots.get(njc, ()) and pend_proj:
                            emit_proj_unit(pend_proj.pop(0))
                    pend_norm.append((hp, ci * 512, av))

            while pend_av:
                flush_av()
            while pend_norm:
                emit_norm_mm(*pend_norm.pop(0))
            pend_proj.extend(range(4 * (IC - 1), 4 * (IC - 1) + 4))
            while pend_proj:
                emit_proj_unit(pend_proj.pop(0), tail=True)
    nc.compile()
    return nc


def _get_nc():
    global _NC
    if _NC is None:
        _NC = _build_nc()
    return _NC


def _in_maps(x, W_qkv, b_qkv, W_proj):
    import ml_dtypes

    bf16 = ml_dtypes.bfloat16
    tri = np.ascontiguousarray(np.triu(np.ones((P, P), dtype=np.float32)).astype(bf16))
    tri2 = np.ascontiguousarray(
        np.concatenate([np.zeros((P, P), np.float32),
                        np.triu(np.ones((P, P), np.float32))], axis=1).astype(bf16)
    )
    sel = np.zeros((P, P), dtype=np.float32)
    sel[D, 0:D] = 1.0
    sel[0, D:P] = 1.0
    vinit = np.zeros((P, TC, 2, D), dtype=bf16)
    vinit[:, :, :, 0] = 1.0
    in_maps = []
    for c in range(8):
        b, g = divmod(c, 4)
        s = slice(HD * g, HD * g + HD)
        sk = slice(C + HD * g, C + HD * g + HD)
        sv = slice(2 * C + HD * g, 2 * C + HD * g + HD)
        in_maps.append({
            "xT": np.ascontiguousarray(x[b].T.astype(bf16)),
            "wq": np.ascontiguousarray(
                W_qkv[:, s].reshape(CC, P, HD).transpose(1, 0, 2).astype(bf16)
            ),
            "wk": np.ascontiguousarray(
                W_qkv[:, sk].reshape(CC, P, HD).transpose(1, 0, 2).astype(bf16)
            ),
            "wv": np.ascontiguousarray(
                W_qkv[:, sv].reshape(CC, P, HD).transpose(1, 0, 2).astype(bf16)
            ),
            "wp": np.ascontiguousarray(
                W_proj[s, :].reshape(2, P, C).transpose(1, 0, 2).astype(bf16)
            ),
            "bq": np.ascontiguousarray(b_qkv[s].reshape(2, P).T),
            "bk": np.ascontiguousarray(b_qkv[sk].reshape(2, P).T),
            "bv": np.ascontiguousarray(
                np.broadcast_to(b_qkv[sv].reshape(2, 2, D), (P, 2, 2, D))
            ),
            "tri": tri,
            "tri2": tri2,
            "sel": sel,
            "vinit": vinit,
        })
    return in_maps


def kernel(x, W_qkv, b_qkv, W_proj, b_proj):
    global LAST_RESULTS
    from concourse import bass_utils

    x = np.asarray(x, dtype=np.float32)
    W_qkv = np.asarray(W_qkv, dtype=np.float32)
    b_qkv = np.asarray(b_qkv, dtype=np.float32)
    W_proj = np.asarray(W_proj, dtype=np.float32)
    b_proj = np.asarray(b_proj, dtype=np.float32)

    nc = _get_nc()
    in_maps = _in_maps(x, W_qkv, b_qkv, W_proj)
    res = bass_utils.run_bass_kernel_spmd(nc, in_maps, core_ids=list(range(8)))
    LAST_RESULTS = res
    ys = []
    for b in range(2):
        y = res.results[4 * b]["out"].astype(np.float64)
        for g in range(1, 4):
            y = y + res.results[4 * b + g]["out"]
        ys.append((y + b_proj).astype(np.float32))
    return np.stack(ys, axis=0)


# revision 21
# speedup vs baseline: 1.0199x; 1.0005x over previous
"""Causal self-attention (B=2, T=2048, C=1024, H=16) on 8 Trainium2 cores.

Sharding: data-parallel over batch (2) x tensor-parallel over heads (4 groups
of 4 heads). Core c handles batch b = c//4, head group g = c%4 (heads 4g..4g+3).
Each core computes its qkv column slice, full causal TxT attention for its 4
heads, and a partial row-parallel projection. Host sums the 4 partial proj
outputs per batch and adds b_proj.

Key performance structure (vs the straightforward two-phase version):
- the PE clock ramps with sustained execution (0.65 -> 1.2 -> 2.4 GHz after
  ~3us of gap-free streaming), so the whole kernel is ONE fused pipeline that
  keeps the PE instruction stream dense: per 512-row i-block ci, emit the qkv
  projections for t-slice ci, then attention for i-block ci, with the
  normalization + output-projection matmuls of earlier blocks woven into the
  score/AV stream as fillers wherever a cross-engine dependency would
  otherwise stall the PE
- score chunk jc's AV matmuls are emitted one step late (after the scores of
  jc+1) so the ACT-engine exp latency is hidden behind PE work
- x and all weights are DMA'd as bf16 (same 1 cycle/row PE rate as fp32r,
  half the HBM traffic / startup latency); on-chip intermediates stay fp32r
  except yT/wp (bf16) - final tolerance is 2e-2, measured error ~1e-3
- softmax denominator comes free from a ones-column embedded in the padded AV
  stationary; both heads' denominator rows are staged into one pre-zeroed
  tile and partition-broadcast with a single K=128 selection matmul, then
  inverted with the fast approximate DVE reciprocal (~5x cheaper than exact)
- PSUM is exactly 8 banks: two pools of [128,2,512]x2 buffers; one rotation
  serves scores / norm-broadcast / projection / qkv tiles, the other the AV
  accumulators
- PSUM->SBUF drains run on the otherwise-idle Pool engine (gpsimd); ACT does
  exp only (it is co-critical with the PE), DVE does bias adds / masks /
  reciprocal / yT scaling
"""

import sys

sys.path.insert(0, "/opt/trn_rl_repo")

import numpy as np

P = 128
T = 2048
C = 1024
D = 64
HPC = 4          # heads per core
HD = HPC * D     # 256 qkv columns per core
CC = C // P      # 8 contraction chunks
TC = T // P      # 16 t-chunks of 128
IC = T // 512    # 4 i-blocks of 512

_NC = None
LAST_RESULTS = None


def _build_nc():
    import concourse.mybir as mybir
    import concourse.tile as tile
    from concourse import bacc
    from contextlib import ExitStack

    dt = mybir.dt
    f32 = dt.float32
    f32r = dt.float32r
    bf16 = dt.bfloat16
    ALU = mybir.AluOpType
    ACTF = mybir.ActivationFunctionType

    nc = bacc.Bacc(
        "TRN2",
        target_bir_lowering=False,
        debug=False,
        enable_asserts=False,
        num_devices=8,
    )

    xT = nc.dram_tensor("xT", [C, T], bf16, kind="ExternalInput").ap()
    wq = nc.dram_tensor("wq", [P, CC, HD], bf16, kind="ExternalInput").ap()
    wk = nc.dram_tensor("wk", [P, CC, HD], bf16, kind="ExternalInput").ap()
    wv = nc.dram_tensor("wv", [P, CC, HD], bf16, kind="ExternalInput").ap()
    wp = nc.dram_tensor("wp", [P, 2, C], bf16, kind="ExternalInput").ap()
    bq = nc.dram_tensor("bq", [P, 2], f32, kind="ExternalInput").ap()
    bk = nc.dram_tensor("bk", [P, 2], f32, kind="ExternalInput").ap()
    bv = nc.dram_tensor("bv", [P, 2, 2, D], f32, kind="ExternalInput").ap()
    tri = nc.dram_tensor("tri", [P, P], bf16, kind="ExternalInput").ap()
    tri2 = nc.dram_tensor("tri2", [P, 2 * P], bf16, kind="ExternalInput").ap()
    sel = nc.dram_tensor("sel", [P, P], f32r, kind="ExternalInput").ap()
    vinit = nc.dram_tensor("vinit", [P, TC, 2, D], bf16, kind="ExternalInput").ap()
    out = nc.dram_tensor("out", [T, C], bf16, kind="ExternalOutput").ap()

    with tile.TileContext(nc) as tc, ExitStack() as ctx:
        persist = ctx.enter_context(tc.tile_pool(name="persist", bufs=1))
        xT_sb = persist.tile([P, CC, T], bf16, name="xTs")
        wq_sb = persist.tile([P, CC, HD], bf16, name="wqs")
        wk_sb = persist.tile([P, CC, HD], bf16, name="wks")
        wv_sb = persist.tile([P, CC, HD], bf16, name="wvs")
        wp_sb = persist.tile([P, 2, C], bf16, name="wps")
        qT_sb = persist.tile([P, 2, T], f32r, name="qT")   # [d%128, hp, t]
        kT_sb = persist.tile([P, 2, T], f32r, name="kT")
        v_sb = persist.tile([P, TC, 2, 2, P], bf16, name="v")
        yT_sb = persist.tile([P, 2, T], bf16, name="yT")
        tri_sb = persist.tile([P, P], bf16, name="tris")
        tri2_sb = persist.tile([P, 2 * P], bf16, name="tri2s")
        sel_sb = persist.tile([P, P], f32r, name="sels")
        dsb = persist.tile([P, 512], f32r, name="dsb")
        bq_sb = persist.tile([P, 2], f32, name="bqs")
        bk_sb = persist.tile([P, 2], f32, name="bks")
        bv_sb = persist.tile([P, 2, 2, D], f32, name="bvs")

        # ---- input DMA, ordered so first-needed data lands first; every
        # large tensor is split across several queue pushes for parallelism.
        # Big streams go on the sync+gpsimd rings; small constants ride the
        # otherwise-idle vector ring so they don't delay x/w.
        xTr = xT.rearrange("(o p) t -> p o t", p=P)
        wqr = wq
        wkr = wk
        wvr = wv
        nc.scalar.dma_start(bk_sb[:], bk)
        nc.scalar.dma_start(bq_sb[:], bq)
        nc.scalar.dma_start(bv_sb[:], bv)
        nc.scalar.dma_start(sel_sb[:], sel)
        for h in range(2):
            cs = slice(4 * h, 4 * h + 4)
            eng = nc.sync if h == 0 else nc.gpsimd
            eng.dma_start(wk_sb[:, cs, :], wkr[:, cs, :])
        for cc in range(CC):
            eng = nc.sync if cc % 2 == 0 else nc.gpsimd
            eng.dma_start(xT_sb[:, cc, 0:512], xTr[:, cc, 0:512])
        for h in range(2):
            cs = slice(4 * h, 4 * h + 4)
            eng = nc.sync if h == 0 else nc.gpsimd
            eng.dma_start(wv_sb[:, cs, :], wvr[:, cs, :])
        for h in range(2):
            cs = slice(4 * h, 4 * h + 4)
            eng = nc.sync if h == 0 else nc.gpsimd
            eng.dma_start(wq_sb[:, cs, :], wqr[:, cs, :])
        nc.scalar.dma_start(v_sb[:, :, :, 0, D:P], vinit)
        nc.scalar.dma_start(v_sb[:, :, :, 1, 0:D], vinit)
        nc.scalar.dma_start(tri_sb[:], tri)
        nc.scalar.dma_start(tri2_sb[:], tri2)
        for tsl in range(1, IC):
            for cc in range(CC):
                eng = nc.sync if cc % 2 == 0 else nc.gpsimd
                eng.dma_start(
                    xT_sb[:, cc, tsl * 512:(tsl + 1) * 512],
                    xTr[:, cc, tsl * 512:(tsl + 1) * 512],
                )
            if tsl == 1:
                nc.sync.dma_start(wp_sb[:, :, 0:512], wp[:, :, 0:512])
                nc.gpsimd.dma_start(wp_sb[:, :, 512:C], wp[:, :, 512:C])
        # zero the denominator staging tile once; each norm only rewrites
        # rows 0 and 64, all other rows must read 0 for the selection matmul
        nc.vector.tensor_scalar_mul(
            dsb[:, :], sel_sb[:, 0:1].to_broadcast([P, 512]), 0.0
        )

        with (
            tc.tile_pool(name="sp", bufs=2, space="PSUM") as sp,
            tc.tile_pool(name="avp", bufs=2, space="PSUM") as avp,
            tc.tile_pool(name="exp", bufs=6) as expool,
            tc.tile_pool(name="oth", bufs=6) as othpool,
        ):
            scale = float(D) ** -0.5
            pend_av = []    # up to two deferred AV pairs (2-deep lookahead)
            pend_norm = []  # (hp, i0, av) awaiting broadcast+reciprocal+scale
            pend_proj = []  # tj indices ready for projection

            def flush_av():
                if not pend_av:
                    return
                hp, jc, njc, av, ex, c0 = pend_av.pop(0)
                for hi in range(2):
                    nc.tensor.matmul(
                        av[:, hi, c0:512],
                        v_sb[:, jc, hp, hi, :],
                        ex[:, hi, c0:512],
                        start=(jc == 0),
                        stop=(jc == njc - 1),
                        skip_group_check=True,
                    )
                if jc == njc - 1:
                    # stage both heads' denominator rows (emitted by the
                    # ones-column of the AV stationary) into the pre-zeroed
                    # dsb; partition-aligned copies (Pool cannot read PSUM)
                    nc.vector.tensor_copy(dsb[D:D + 1, :], av[D:D + 1, 0, :])
                    nc.vector.tensor_copy(dsb[0:1, :], av[0:1, 1, :])

            def emit_s(ci, hp, jc):
                i0 = ci * 512
                diag = jc >= 4 * ci
                o = (jc - 4 * ci) if diag else 0
                c0 = 2 * P if diag and o == 3 else o * P
                sps = sp.tile([P, 2, 512], f32, tag="s")
                for hi in range(2):
                    bp = D * hi
                    nc.tensor.matmul(
                        sps[:, hi, c0:512],
                        kT_sb[bp:bp + D, hp, jc * P:(jc + 1) * P],
                        qT_sb[bp:bp + D, hp, i0 + c0:i0 + 512],
                        start=True,
                        stop=True,
                        skip_group_check=True,
                    )
                ex = expool.tile([P, 2, 512], bf16, tag="ex")
                nc.scalar.activation(
                    ex[:, :, c0:512], sps[:, :, c0:512], ACTF.Exp, scale=scale
                )
                if diag and o == 3:
                    # cols 256-383 fully masked, 384-511 triangular
                    nc.vector.tensor_tensor(
                        ex[:, :, c0:512],
                        ex[:, :, c0:512],
                        tri2_sb[:, None, :].to_broadcast([P, 2, 2 * P]),
                        ALU.mult,
                    )
                elif diag:
                    nc.vector.tensor_tensor(
                        ex[:, :, c0:c0 + P],
                        ex[:, :, c0:c0 + P],
                        tri_sb[:, None, :].to_broadcast([P, 2, P]),
                        ALU.mult,
                    )
                return ex, c0

            def emit_norm_mm(hp, i0, av):
                # partition-broadcast both denominator rows with one K=128
                # selection matmul, invert with the fast approx reciprocal
                # (denominators are >= ~1, well inside its safe range),
                # scale the AV values into yT
                bps = sp.tile([P, 2, 512], f32, tag="s")
                nc.tensor.matmul(
                    bps[:, 0, :], sel_sb[:], dsb[:],
                    start=True, stop=True, skip_group_check=True,
                )
                rec = othpool.tile([P, 512], f32, tag="rec")
                nc.vector.reciprocal_approx_fast(rec[:], bps[:, 0, :])
                nc.vector.tensor_tensor(
                    yT_sb[0:D, hp, i0:i0 + 512], av[0:D, 0, :], rec[0:D, :],
                    ALU.mult,
                )
                nc.vector.tensor_tensor(
                    yT_sb[D:P, hp, i0:i0 + 512], av[D:P, 1, :], rec[D:P, :],
                    ALU.mult,
                )

            def emit_proj_unit(tj, tail=False):
                pps = sp.tile([P, 2, 512], f32, tag="s")
                ot = othpool.tile([P, C], bf16, tag="ot")
                for co in range(2):
                    for dc in range(2):
                        nc.tensor.matmul(
                            pps[:, co, :],
                            yT_sb[:, dc, tj * P:(tj + 1) * P],
                            wp_sb[:, dc, co * 512:(co + 1) * 512],
                            start=(dc == 0),
                            stop=(dc == 1),
                        )
                nc.vector.tensor_copy(
                    ot[:].rearrange("p (co n) -> p co n", co=2), pps[:]
                )
                # two stores per unit on alternating rings: halves the
                # per-queue drain (descriptor-rate-bound) without flooding
                # the rings with triggers; tail units split 4-way since
                # nothing overlaps the final drain
                three = (nc.sync, nc.gpsimd, nc.scalar)
                rings = ((three[tj % 3], three[(tj + 1) % 3])
                         if tail else (nc.sync, nc.gpsimd))
                for ph in range(2):
                    rings[ph].dma_start(
                        out[tj * P + ph * D:tj * P + (ph + 1) * D, :],
                        ot[ph * D:(ph + 1) * D, :],
                    )

            for ci in range(IC):
                # ---- qkv projections for t-slice ci ----
                # group order k -> v -> q: each group's PSUM drains on DVE
                # while the next group's matmuls stream, and the S(0) gate
                # (q bias add) is emitted per-co right behind its matmuls
                vdrains = []
                for gi, (w_s, b_s, dest) in enumerate(
                    ((wk_sb, bk_sb, kT_sb), (wq_sb, bq_sb, qT_sb))
                ):
                    ps = (avp if gi == 0 else sp).tile(
                        [P, 2, 512], f32, tag="av" if gi == 0 else "s"
                    )
                    for co in range(2):
                        for cc in range(CC):
                            nc.tensor.matmul(
                                ps[:, co, :],
                                w_s[:, cc, co * P:(co + 1) * P],
                                xT_sb[:, cc, ci * 512:(ci + 1) * 512],
                                start=(cc == 0),
                                stop=(cc == CC - 1),
                            )
                        if gi == 0:
                            # previous i-block's trailing AV pairs: their
                            # exps have had a full matmul group to complete
                            flush_av()
                        nc.vector.tensor_tensor(
                            dest[:, co, ci * 512:(ci + 1) * 512],
                            ps[:, co, :],
                            b_s[:, co:co + 1].to_broadcast([P, 512]),
                            ALU.add,
                        )
                    if gi == 0:
                        ps = sp.tile([P, 2, 512], f32, tag="s")
                        for tj4 in range(4):
                            tj = 4 * ci + tj4
                            pr = ps[:, tj4 // 2,
                                    (tj4 % 2) * 256:(tj4 % 2) * 256 + 256]
                            for cc in range(CC):
                                nc.tensor.matmul(
                                    pr,
                                    xT_sb[:, cc, tj * P:(tj + 1) * P],
                                    wv_sb[:, cc, :],
                                    start=(cc == 0),
                                    stop=(cc == CC - 1),
                                    skip_group_check=True,
                                )
                            psv = pr.rearrange(
                                "p (hp hi d) -> p hp hi d", hi=2, d=D
                            )
                            vdrains.append((tj, psv))
                # v drains go on DVE after the q bias adds: their AV
                # consumers are the diagonal chunks, several jc away
                for tj, psv in vdrains:
                    nc.vector.tensor_tensor(
                        v_sb[:, tj, :, 0, 0:D], psv[:, :, 0, :], bv_sb[:, :, 0, :],
                        ALU.add,
                    )
                    nc.vector.tensor_tensor(
                        v_sb[:, tj, :, 1, D:P], psv[:, :, 1, :], bv_sb[:, :, 1, :],
                        ALU.add,
                    )
                if pend_norm:
                    emit_norm_mm(*pend_norm.pop(0))
                    if ci >= 1:
                        pend_proj.extend(range(4 * (ci - 1), 4 * (ci - 1) + 4))

                # ---- attention for i-block ci ----
                njc = 4 * (ci + 1)
                for hp in range(2):
                    av = avp.tile([P, 2, 512], f32, tag="av")
                    for jc in range(njc):
                        if len(pend_av) >= 2:
                            flush_av()
                        ex, c0 = emit_s(ci, hp, jc)
                        pend_av.append((hp, jc, njc, av, ex, c0))
                        if jc == 3 and pend_norm:
                            emit_norm_mm(*pend_norm.pop(0))
                        pslots = {8: (4, 6), 12: (5, 9), 16: (6, 11)}
                        if jc in pslots.get(njc, ()) and pend_proj:
                            emit_proj_unit(pend_proj.pop(0))
                    pend_norm.append((hp, ci * 512, av))

            while pend_av:
                flush_av()
            while pend_norm:
                emit_norm_mm(*pend_norm.pop(0))
            pend_proj.extend(range(4 * (IC - 1), 4 * (IC - 1) + 4))
            while pend_proj:
                emit_proj_unit(pend_proj.pop(0), tail=True)
    nc.compile()
    return nc


def _get_nc():
    global _NC
    if _NC is None:
        _NC = _build_nc()
    return _NC


def _in_maps(x, W_qkv, b_qkv, W_proj):
    import ml_dtypes

    bf16 = ml_dtypes.bfloat16
    tri = np.ascontiguousarray(np.triu(np.ones((P, P), dtype=np.float32)).astype(bf16))
    tri2 = np.ascontiguousarray(
        np.concatenate([np.zeros((P, P), np.float32),
                        np.triu(np.ones((P, P), np.float32))], axis=1).astype(bf16)
    )
    sel = np.zeros((P, P), dtype=np.float32)
    sel[D, 0:D] = 1.0
    sel[0, D:P] = 1.0
    vinit = np.zeros((P, TC, 2, D), dtype=bf16)
    vinit[:, :, :, 0] = 1.0
    in_maps = []
    for c in range(8):
        b, g = divmod(c, 4)
        s = slice(HD * g, HD * g + HD)
        sk = slice(C + HD * g, C + HD * g + HD)
        sv = slice(2 * C + HD * g, 2 * C + HD * g + HD)
        in_maps.append({
            "xT": np.ascontiguousarray(x[b].T.astype(bf16)),
            "wq": np.ascontiguousarray(
                W_qkv[:, s].reshape(CC, P, HD).transpose(1, 0, 2).astype(bf16)
            ),
            "wk": np.ascontiguousarray(
                W_qkv[:, sk].reshape(CC, P, HD).transpose(1, 0, 2).astype(bf16)
            ),
            "wv": np.ascontiguousarray(
                W_qkv[:, sv].reshape(CC, P, HD).transpose(1, 0, 2).astype(bf16)
            ),
            "wp": np.ascontiguousarray(
                W_proj[s, :].reshape(2, P, C).transpose(1, 0, 2).astype(bf16)
            ),
            "bq": np.ascontiguousarray(b_qkv[s].reshape(2, P).T),
            "bk": np.ascontiguousarray(b_qkv[sk].reshape(2, P).T),
            "bv": np.ascontiguousarray(
                np.broadcast_to(b_qkv[sv].reshape(2, 2, D), (P, 2, 2, D))
            ),
            "tri": tri,
            "tri2": tri2,
            "sel": sel,
            "vinit": vinit,
        })
    return in_maps


def kernel(x, W_qkv, b_qkv, W_proj, b_proj):
    global LAST_RESULTS
    from concourse import bass_utils

    x = np.asarray(x, dtype=np.float32)
    W_qkv = np.asarray(W_qkv, dtype=np.float32)
    b_qkv = np.asarray(b_qkv, dtype=np.float32)
    W_proj = np.asarray(W_proj, dtype=np.float32)
    b_proj = np.asarray(b_proj, dtype=np.float32)

    nc = _get_nc()
    in_maps = _in_maps(x, W_qkv, b_qkv, W_proj)
    res = bass_utils.run_bass_kernel_spmd(nc, in_maps, core_ids=list(range(8)))
    LAST_RESULTS = res
    ys = []
    for b in range(2):
        y = res.results[4 * b]["out"].astype(np.float64)
        for g in range(1, 4):
            y = y + res.results[4 * b + g]["out"]
        ys.append((y + b_proj).astype(np.float32))
    return np.stack(ys, axis=0)
